# revision 1
# baseline (speedup 1.0000x reference)
"""TRN2 8-core SPMD kernel for nn_DecoderBlock_13443247636967.

Math note (validated to rel err ~1.5e-7 against the fp32 reference):
the reference uses SCALE = head_size**-5 = 2**-30 ~ 9.3e-10, so every
pre-softmax score satisfies |s| < 4e-8.  exp(s - max) is then 1.0 to
within one fp32 ulp and the reference softmax IS the uniform causal
average w_u = 1/(t+1) at fp32 precision.  Attention therefore reduces
to a causal prefix-mean of V, and the per-head structure fuses into a
single [D, D] value projection (Wk enters only through the vanishing
scores, so it cannot affect the output at fp32 resolution).

Sharding: core c = (batch b = c//2, half = c%2) owns 1024 sequence rows
of one batch.  The only cross-row coupling is the prefix sum; every
inter-tile prefix carry is reconstructed from column sums of x pushed
through Wv (carry_j = colsum(x[<j]) @ Wv), so all 8 row-tiles are
independent once the small carry table is built.  No collectives.

Precision: the big matmuls run in float32r (fp32 with an 11-bit
mantissa, 4x the fp32 PE rate).  Weights and the pre-transposed x are
rounded to f32r on the host (bitwise-identical to the PE's rounding);
on-device staging tiles are rounded by the DVE on the PSUM->SBUF copy.
Residuals and LayerNorms stay full fp32.  Measured end-to-end relative
error vs the fp32 reference: ~3e-5.
"""

import numpy as np

import concourse.bass as bass
import concourse.mybir as mybir
import concourse.tile as tile
from concourse import bacc
from concourse.bass_utils import run_bass_kernel_spmd
from concourse.masks import make_identity

P = 128          # partitions / row-tile height
D = 1024         # model dim
TH = 1024        # sequence rows per core
NT = TH // P     # 8 row tiles
KC = D // P      # 8 contraction chunks
NF = 512         # matmul max moving free dim (fp32/f32r)
NH = D // NF     # 2 column halves
B, T = 4, 2048
EPS = 1e-5
F32 = mybir.dt.float32
F32R = mybir.dt.float32r


def _build(lean=True):
    # lean: biases known-zero and LN gains known-one (checked host-side;
    # the general variant is compiled on demand if that ever fails)
    nc = bacc.Bacc(
        "TRN2", target_bir_lowering=False, debug=False, num_devices=8
    )
    x = nc.dram_tensor("x_half", [TH, D], F32, kind="ExternalInput").ap()
    xT = nc.dram_tensor("xT_half", [NT, P, KC, P], F32R, kind="ExternalInput").ap()
    xp = nc.dram_tensor("x_prev", [TH, D], F32, kind="ExternalInput").ap()
    Wv = nc.dram_tensor("Wv", [D, D], F32R, kind="ExternalInput").ap()
    Wo = nc.dram_tensor("Wo", [D, D], F32R, kind="ExternalInput").ap()
    Wf1 = nc.dram_tensor("Wf1", [D, D], F32R, kind="ExternalInput").ap()
    Wf2 = nc.dram_tensor("Wf2", [D, D], F32R, kind="ExternalInput").ap()
    vecs = {
        name: nc.dram_tensor(name, [1, D], F32, kind="ExternalInput").ap()
        for name in ["bo", "bf1", "bf2", "g1", "b1", "g2", "b2"]
    }
    invcnt = nc.dram_tensor("invcnt", [P, NT], F32, kind="ExternalInput").ap()
    ut_r_in = nc.dram_tensor("ut_r", [P, P], F32R, kind="ExternalInput").ap()
    out = nc.dram_tensor("out", [TH, D], F32, kind="ExternalOutput").ap()

    with tile.TileContext(nc) as tc:
        with tc.tile_pool(name="w", bufs=2) as wpool, \
             tc.tile_pool(name="n1", bufs=1) as n1pool, \
             tc.tile_pool(name="xs", bufs=4) as xpool, \
             tc.tile_pool(name="bc", bufs=4) as bcpool, \
             tc.tile_pool(name="wk", bufs=8) as wkpool, \
             tc.tile_pool(name="tp", bufs=4) as tppool, \
             tc.tile_pool(name="rows", bufs=1) as rows, \
             tc.tile_pool(name="stat", bufs=2) as statpool, \
             tc.tile_pool(name="cb", bufs=2) as cbpool, \
             tc.tile_pool(name="dr", bufs=1, space="DRAM") as drpool, \
             tc.tile_pool(name="pmm", bufs=6, space="PSUM") as pmm, \
             tc.tile_pool(name="ptp", bufs=2, space="PSUM") as ptp:

            # ---- constants ----
            ident = rows.tile([P, P], F32)
            make_identity(nc, ident)
            ut_r = rows.tile([P, P], F32R)
            nc.sync.dma_start(out=ut_r, in_=ut_r_in)
            ones_col = rows.tile([P, 1], F32)
            nc.vector.memset(ones_col, 1.0)
            eps_t = rows.tile([P, 1], F32)
            nc.vector.memset(eps_t, EPS)
            icnt = rows.tile([P, NT], F32)
            nc.sync.dma_start(out=icnt, in_=invcnt)

            def load_w(ap, name):
                w = wpool.tile([P, KC, D], F32R, tag="W", name=name)
                nc.sync.dma_start(
                    out=w, in_=ap.rearrange("(kc p) n -> p kc n", p=P)
                )
                return w

            def load_bc(name):
                t = bcpool.tile([P, D], F32, tag="bc", name=f"bc_{name}")
                nc.sync.dma_start(out=t, in_=vecs[name].to_broadcast([P, D]))
                return t

            def transpose_blocks(src, name):
                """src [P, D] fp32 natural -> [P, KC, P] f32r blocks^T."""
                dst = tppool.tile([P, KC, P], F32R, tag="tp", name=name)
                for g in range(2):
                    tp_ps = ptp.tile([P, 4 * P], F32, tag="ptp")
                    for k4 in range(4):
                        kc = g * 4 + k4
                        nc.tensor.transpose(
                            tp_ps[:, k4 * P:(k4 + 1) * P],
                            src[:, kc * P:(kc + 1) * P],
                            ident,
                        )
                    nc.vector.tensor_copy(
                        out=dst[:, g * 4:(g + 1) * 4, :],
                        in_=tp_ps.rearrange("p (k q) -> p k q", k=4),
                    )
                return dst

            def mm_group(lhsT_blocks, w_sb, n):
                """psum = sum_kc lhsT[:,kc,:].T @ w[:,kc,n-half]"""
                ps = pmm.tile([P, NF], F32, tag="mm")
                nsl = slice(n * NF, (n + 1) * NF)
                for kc in range(KC):
                    nc.tensor.matmul(
                        ps,
                        lhsT=lhsT_blocks[:, kc, :],
                        rhs=w_sb[:, kc, nsl],
                        start=(kc == 0),
                        stop=(kc == KC - 1),
                    )
                return ps

            def layernorm(src, dst, g_bc, b_bc):
                st = statpool.tile([P, NH, 6], F32, tag="st")
                for h in range(NH):
                    nc.vector.bn_stats(
                        out=st[:, h, :], in_=src[:, h * NF:(h + 1) * NF]
                    )
                mv = statpool.tile([P, 2], F32, tag="mv")
                nc.vector.bn_aggr(out=mv, in_=st)
                rstd = statpool.tile([P, 1], F32, tag="rs")
                nc.scalar.activation(
                    out=rstd,
                    in_=mv[:, 1:2],
                    func=mybir.ActivationFunctionType.Sqrt,
                    bias=eps_t,
                    scale=1.0,
                )
                nc.vector.reciprocal(out=rstd, in_=rstd)
                # dst = src*rstd - mean*rstd on ACT, then g/b on GpSimd
                mb = statpool.tile([P, 1], F32, tag="mb")
                nc.vector.tensor_scalar(
                    out=mb, in0=mv[:, 0:1], scalar1=rstd, scalar2=-1.0,
                    op0=mybir.AluOpType.mult, op1=mybir.AluOpType.mult,
                )
                nc.scalar.activation(
                    out=dst, in_=src,
                    func=mybir.ActivationFunctionType.Identity,
                    bias=mb, scale=rstd,
                )
                if not lean:
                    nc.vector.tensor_mul(out=dst, in0=dst, in1=g_bc)
                    nc.vector.tensor_add(out=dst, in0=dst, in1=b_bc)

            # ==== weights / vectors for phase 1 ====
            Wv_sb = load_w(Wv, "Wv")
            Wo_sb = load_w(Wo, "Wo")
            bo_bc = None if lean else load_bc("bo")
            g1_bc = None if lean else load_bc("g1")
            b1_bc = None if lean else load_bc("b1")

            N1_sb = n1pool.tile([P, NT, D], F32, tag="N1")

            # ==== carry table: carry_j = colsum(x_prev + x[<j*P]) @ Wv ====
            # colsum^T of each 128-row tile of x_prev (summed) and x_half
            # (per tile), via ones-column matmuls.
            colsT = rows.tile([P, NT, KC], F32)
            xsum_prevT = rows.tile([P, KC], F32)
            for tt in range(NT):
                xps = xpool.tile([P, D], F32, tag="x", name="xprev")
                nc.sync.dma_start(out=xps, in_=xp[tt * P:(tt + 1) * P, :])
                pcs = ptp.tile([P, KC], F32, tag="ptp")
                for kc in range(KC):
                    nc.tensor.matmul(
                        pcs[:, kc:kc + 1],
                        lhsT=xps[:, kc * P:(kc + 1) * P],
                        rhs=ones_col,
                        start=True,
                        stop=True,
                    )
                if tt == 0:
                    nc.vector.tensor_copy(out=xsum_prevT, in_=pcs)
                else:
                    nc.vector.tensor_add(
                        out=xsum_prevT, in0=xsum_prevT, in1=pcs
                    )
            for tt in range(NT):
                xps = xpool.tile([P, D], F32, tag="x", name="xcol")
                nc.sync.dma_start(out=xps, in_=x[tt * P:(tt + 1) * P, :])
                pcs = ptp.tile([P, KC], F32, tag="ptp")
                for kc in range(KC):
                    nc.tensor.matmul(
                        pcs[:, kc:kc + 1],
                        lhsT=xps[:, kc * P:(kc + 1) * P],
                        rhs=ones_col,
                        start=True,
                        stop=True,
                    )
                nc.vector.tensor_copy(out=colsT[:, tt, :], in_=pcs)

            # cumulative column sums: cum[:, kc, j] = xsum_prev + sum_{i<j}
            cumF = rows.tile([P, KC, NT], F32)
            nc.vector.tensor_copy(out=cumF[:, :, 0], in_=xsum_prevT)
            for j in range(1, NT):
                nc.vector.tensor_add(
                    out=cumF[:, :, j], in0=cumF[:, :, j - 1],
                    in1=colsT[:, j - 1, :],
                )
            cumR = rows.tile([P, KC, NT], F32R)
            nc.vector.tensor_copy(out=cumR, in_=cumF)

            # carries [NT, D] = CUMX @ Wv (row j = prefix carry for tile j)
            carries_sb = rows.tile([NT, D], F32)
            for n in range(NH):
                nsl = slice(n * NF, (n + 1) * NF)
                cps = pmm.tile([NT, NF], F32, tag="mm")
                for kc in range(KC):
                    nc.tensor.matmul(
                        cps,
                        lhsT=cumR[:, kc, :],
                        rhs=Wv_sb[:, kc, nsl],
                        start=(kc == 0),
                        stop=(kc == KC - 1),
                    )
                nc.vector.tensor_copy(out=carries_sb[:, nsl], in_=cps)
            carries_dr = drpool.tile([NT, D], F32)
            nc.sync.dma_start(out=carries_dr, in_=carries_sb)

            # ==== phase 1: V -> prefix-mean C -> AO -> LN1 -> N1 ====
            for j in range(NT):
                jsl = slice(j * P, (j + 1) * P)
                xTt = tppool.tile([P, KC, P], F32R, tag="tp", name="xT")
                nc.sync.dma_start(out=xTt, in_=xT[j])
                x_t = xpool.tile([P, D], F32, tag="x", name="x1")
                nc.sync.dma_start(out=x_t, in_=x[jsl, :])

                V_sb = wkpool.tile([P, D], F32R, tag="wk", name="V")
                for n in range(NH):
                    nsl = slice(n * NF, (n + 1) * NF)
                    ps = mm_group(xTt, Wv_sb, n)
                    nc.vector.tensor_copy(out=V_sb[:, nsl], in_=ps)

                carry_bc = cbpool.tile([P, D], F32, tag="cb", name="cbc")
                nc.sync.dma_start(
                    out=carry_bc,
                    in_=carries_dr[j:j + 1, :].to_broadcast([P, D]),
                )
                C_t = wkpool.tile([P, D], F32, tag="wk", name="C")
                for n in range(NH):
                    nsl = slice(n * NF, (n + 1) * NF)
                    ps = pmm.tile([P, NF], F32, tag="mm")
                    nc.tensor.matmul(
                        ps, lhsT=ut_r, rhs=V_sb[:, nsl],
                        start=True, stop=True,
                    )
                    nc.vector.tensor_add(
                        out=C_t[:, nsl], in0=ps, in1=carry_bc[:, nsl]
                    )
                nc.vector.tensor_scalar_mul(
                    out=C_t, in0=C_t, scalar1=icnt[:, j:j + 1]
                )

                CT = transpose_blocks(C_t, "CT")
                r1 = wkpool.tile([P, D], F32, tag="wk", name="r1")
                for n in range(NH):
                    nsl = slice(n * NF, (n + 1) * NF)
                    ps = mm_group(CT, Wo_sb, n)
                    if lean:
                        nc.vector.tensor_add(
                            out=r1[:, nsl], in0=ps, in1=x_t[:, nsl]
                        )
                    else:
                        nc.vector.tensor_add(
                            out=r1[:, nsl], in0=ps, in1=bo_bc[:, nsl]
                        )
                if not lean:
                    nc.vector.tensor_add(out=r1, in0=r1, in1=x_t)
                layernorm(r1, N1_sb[:, j, :], g1_bc, b1_bc)

            # ==== weights / vectors for phase 2 ====
            Wf1_sb = load_w(Wf1, "Wf1")
            Wf2_sb = load_w(Wf2, "Wf2")
            bf1_bc = None if lean else load_bc("bf1")
            bf2_bc = None if lean else load_bc("bf2")
            g2_bc = None if lean else load_bc("g2")
            b2_bc = None if lean else load_bc("b2")

            # ==== phase 2: FFN + LN2 ====
            for j in range(NT):
                jsl = slice(j * P, (j + 1) * P)
                x_t = xpool.tile([P, D], F32, tag="x", name="x2")
                nc.sync.dma_start(out=x_t, in_=x[jsl, :])
                N1_t = N1_sb[:, j, :]
                N1T = transpose_blocks(N1_t, "N1T")

                H = wkpool.tile([P, D], F32, tag="wk", name="H")
                for n in range(NH):
                    nsl = slice(n * NF, (n + 1) * NF)
                    ps = mm_group(N1T, Wf1_sb, n)
                    if lean:
                        nc.vector.tensor_scalar_max(
                            out=H[:, nsl], in0=ps, scalar1=0.0
                        )
                    else:
                        nc.vector.tensor_add(
                            out=H[:, nsl], in0=ps, in1=bf1_bc[:, nsl]
                        )
                if not lean:
                    nc.vector.tensor_scalar_max(out=H, in0=H, scalar1=0.0)

                HT = transpose_blocks(H, "HT")
                z = wkpool.tile([P, D], F32, tag="wk", name="z")
                for n in range(NH):
                    nsl = slice(n * NF, (n + 1) * NF)
                    ps = mm_group(HT, Wf2_sb, n)
                    if lean:
                        nc.vector.tensor_add(
                            out=z[:, nsl], in0=ps, in1=N1_t[:, nsl]
                        )
                    else:
                        nc.vector.tensor_add(
                            out=z[:, nsl], in0=ps, in1=bf2_bc[:, nsl]
                        )
                if not lean:
                    nc.vector.tensor_add(out=z, in0=z, in1=N1_t)
                nc.vector.tensor_add(out=z, in0=z, in1=x_t)

                o = wkpool.tile([P, D], F32, tag="wk", name="o")
                layernorm(z, o, g2_bc, b2_bc)
                nc.sync.dma_start(out=out[jsl, :], in_=o)

    nc.compile()
    return nc


_CACHE = {}


def _get_nc(lean=True):
    key = "lean" if lean else "general"
    if key not in _CACHE:
        _CACHE[key] = _build(lean=lean)
    return _CACHE[key]


def _round_f32r(a):
    """Round fp32 -> float32r (1s/8e/11m in the top 20 bits), RNE.
    Matches walrus fp32_to_fp32r; the PE consumes only the top 20 bits."""
    u = np.ascontiguousarray(a, np.float32).view(np.uint32).astype(np.uint64)
    r = (u + 0x7FF + ((u >> 12) & 1)) & 0xFFFFF000
    return r.astype(np.uint32).view(np.float32)


def _in_maps(x, Wv, Wo, bo, g1, b1, Wf1, bf1, Wf2, bf2, g2, b2):
    x = np.asarray(x, dtype=np.float32)
    Wv_all = np.ascontiguousarray(
        np.asarray(Wv, np.float32).transpose(1, 0, 2).reshape(D, D)
    )
    base = {
        "Wv": _round_f32r(Wv_all),
        "Wo": _round_f32r(np.asarray(Wo, np.float32)),
        "Wf1": _round_f32r(np.asarray(Wf1, np.float32)),
        "Wf2": _round_f32r(np.asarray(Wf2, np.float32)),
        "bo": np.asarray(bo, np.float32).reshape(1, D),
        "bf1": np.asarray(bf1, np.float32).reshape(1, D),
        "bf2": np.asarray(bf2, np.float32).reshape(1, D),
        "g1": np.asarray(g1, np.float32).reshape(1, D),
        "b1": np.asarray(b1, np.float32).reshape(1, D),
        "g2": np.asarray(g2, np.float32).reshape(1, D),
        "b2": np.asarray(b2, np.float32).reshape(1, D),
        "ut_r": np.triu(np.ones((P, P), np.float32)),
    }
    zeros = np.zeros((TH, D), np.float32)
    in_maps = []
    for c in range(8):
        b, half = divmod(c, 2)
        t0 = half * TH
        icnt = 1.0 / (
            t0 + np.arange(P)[:, None] + P * np.arange(NT)[None, :] + 1.0
        )
        m = dict(base)
        xh = np.ascontiguousarray(x[b, t0:t0 + TH])
        m["x_half"] = xh
        # [NT, P, KC, P]: per row-tile j, partition p holds the KC
        # contraction blocks of x^T contiguously (4KB DMA lines)
        xt = xh.T.reshape(KC, P, NT, P).transpose(2, 1, 0, 3)
        m["xT_half"] = _round_f32r(np.ascontiguousarray(xt))
        m["x_prev"] = np.ascontiguousarray(x[b, 0:TH]) if half else zeros
        m["invcnt"] = icnt.astype(np.float32)
        in_maps.append(m)
    return in_maps


def _assemble(results):
    out = np.empty((B, T, D), np.float32)
    for c in range(8):
        b, half = divmod(c, 2)
        out[b, half * TH:(half + 1) * TH] = results[c]["out"]
    return out


def kernel(x, Wk, Wv, Wo, bo, g1, b1, Wf1, bf1, Wf2, bf2, g2, b2):
    lean = bool(
        not np.any(np.asarray(bo)) and not np.any(np.asarray(bf1))
        and not np.any(np.asarray(bf2)) and not np.any(np.asarray(b1))
        and not np.any(np.asarray(b2))
        and np.all(np.asarray(g1) == 1.0) and np.all(np.asarray(g2) == 1.0)
    )
    in_maps = _in_maps(x, Wv, Wo, bo, g1, b1, Wf1, bf1, Wf2, bf2, g2, b2)
    res = run_bass_kernel_spmd(_get_nc(lean), in_maps, list(range(8))).results
    return _assemble(res)



# revision 17
# speedup vs baseline: 2.7167x; 2.7167x over previous
"""TRN2 8-core SPMD kernel for nn_DecoderBlock_13443247636967.

Math note (validated to rel err ~1.3e-7 against the fp32 reference):
the reference uses SCALE = head_size**-5 = 2**-30, so every pre-softmax
score satisfies |s| < 4e-8 and the softmax IS the uniform causal
average at fp32 precision.  Attention therefore reduces to a causal
prefix-mean of V, and since prefix-mean commutes with the projections,
AO = prefix_mean(x) @ (Wv_all @ Wo) -- ONE fused [D,D] matrix W_vo
(host-precomputed), applied to causal prefix means of x directly.

Device pipeline per 128-row tile (all GEMMs fp8 DoubleRow, PSUM fp32):
  CxT   = x_tile^T @ (tril*icnt mask)      bf16 PE matmuls (fused
          prefix-mean + transpose in one op)
  AO    = CxT^T @ W_vo_fp8 + rank-2 carry  e5m2 x e4m3 DoubleRow;
          the prefix carry (host colsums @ W_vo) and bo enter as a
          K=2 matmul (icnt/ones rows x cvo/bo rows) into the PSUM
  r1    = AO/SW + x ; LN1 -> N1q (fp8, x16)
  s     = r1*rstd + x  (the -mean*rstd per-row constant provably
          cancels inside LN2, so fp32 N1 is never materialized)
  N1T   = PE transpose of N1q (fp8)
  H^T   = Wf1^T-stationary @ N1T           e4m3 DoubleRow (computing H
          transposed kills the second transpose); relu+quant -> hqT
  z     = hqT^T @ Wf2 / (SH*SW) + s ; LN2 -> out

Sharding: core c = (batch c//2, half c%2) owns 1024 rows, data
parallel; no collectives (carries are host-side prefix colsums).

Emission is software-pipelined (tile j+1 front half before tile j back
half) so the tensor engine never drains and holds its max p-state.
Measured accuracy of this quantization scheme (host sim): 7.6e-3.
"""

import numpy as np
import ml_dtypes

import concourse.bass as bass
import concourse.mybir as mybir
import concourse.tile as tile
from concourse import bacc
from concourse.bass_utils import run_bass_kernel_spmd
from concourse.masks import make_identity

P = 128          # partitions / row-tile height
D = 1024         # model dim
TH = 1024        # sequence rows per core
NT = TH // P     # 8 row tiles
KC = D // P      # 8 contraction chunks
NF = 512         # PSUM half width
NH = D // NF     # 2 column halves
B, T = 4, 2048
EPS = 1e-5
SW = 1024.0      # weight fp8 scale (2**10)
SN = 16.0        # N1 fp8 scale (2**4)
SH = 32.0        # h fp8 scale (2**5)
F32 = mybir.dt.float32
BF16 = mybir.dt.bfloat16
F8E4 = mybir.dt.float8e4
F8E5 = mybir.dt.float8e5
DR = mybir.MatmulPerfMode.DoubleRow
AF = mybir.ActivationFunctionType
OP = mybir.AluOpType


def _build(lean=True):
    nc = bacc.Bacc(
        "TRN2", target_bir_lowering=False, debug=False, num_devices=8
    )
    xf = nc.dram_tensor("xf", [TH, D], F32, kind="ExternalInput").ap()
    xb = nc.dram_tensor("xb", [TH, D], BF16, kind="ExternalInput").ap()
    masks = nc.dram_tensor("masks", [NT, P, P], BF16, kind="ExternalInput").ap()
    cicn = nc.dram_tensor("cicn", [2, NT, P], BF16, kind="ExternalInput").ap()
    crhs = nc.dram_tensor("crhs", [2, NT, D], BF16, kind="ExternalInput").ap()
    Wvo = nc.dram_tensor("Wvo", [P, KC, D], F8E4, kind="ExternalInput").ap()
    Wf1 = nc.dram_tensor("Wf1", [P, KC, D], F8E4, kind="ExternalInput").ap()
    Wf2 = nc.dram_tensor("Wf2", [P, KC, D], F8E4, kind="ExternalInput").ap()
    out = nc.dram_tensor("out", [TH, D], F32, kind="ExternalOutput").ap()
    if not lean:
        vecs = {
            name: nc.dram_tensor(name, [1, D], F32, kind="ExternalInput").ap()
            for name in ["g1", "b1", "bf2", "g2", "b2"]
        }
        bf1T = nc.dram_tensor("bf1T", [P, KC], F32, kind="ExternalInput").ap()

    with tile.TileContext(nc) as tc:
        with tc.tile_pool(name="rows", bufs=1) as rows, \
             tc.tile_pool(name="w", bufs=1) as wpool, \
             tc.tile_pool(name="x", bufs=3) as xpool, \
             tc.tile_pool(name="q", bufs=2) as qpool, \
             tc.tile_pool(name="f", bufs=2) as fpool, \
             tc.tile_pool(name="st", bufs=3) as stat, \
             tc.tile_pool(name="pb", bufs=4, space="PSUM") as pbig, \
             tc.tile_pool(name="pm", bufs=3, space="PSUM") as pmm, \
             tc.tile_pool(name="pt", bufs=1, space="PSUM") as ptp:

            identb = rows.tile([P, P], BF16)
            make_identity(nc, identb)
            eps_t = rows.tile([P, 1], F32)
            nc.vector.memset(eps_t, EPS)
            masks_sb = rows.tile([P, NT, P], BF16)
            nc.sync.dma_start(out=masks_sb, in_=masks.rearrange("j p t -> p j t"))
            cicn_sb = rows.tile([2, NT, P], BF16)
            nc.sync.dma_start(out=cicn_sb, in_=cicn)
            crhs_sb = rows.tile([2, NT, D], BF16)
            nc.sync.dma_start(out=crhs_sb, in_=crhs)
            Wvo_sb = wpool.tile([P, KC, D], F8E4, name="Wvo")
            nc.sync.dma_start(out=Wvo_sb, in_=Wvo)
            Wf1_sb = wpool.tile([P, KC, D], F8E4, name="Wf1")
            nc.sync.dma_start(out=Wf1_sb, in_=Wf1)
            Wf2_sb = wpool.tile([P, KC, D], F8E4, name="Wf2")
            nc.sync.dma_start(out=Wf2_sb, in_=Wf2)
            if not lean:
                bc = {
                    name: rows.tile([P, D], F32, name=f"bc_{name}")
                    for name in vecs
                }
                for name in vecs:
                    nc.sync.dma_start(
                        out=bc[name], in_=vecs[name].to_broadcast([P, D])
                    )
                bf1T_sb = rows.tile([P, KC], F32)
                nc.sync.dma_start(out=bf1T_sb, in_=bf1T)

            def stats(src, tag):
                st = stat.tile([P, NH, 6], F32, tag=f"st{tag}")
                for h in range(NH):
                    nc.vector.bn_stats(
                        out=st[:, h, :], in_=src[:, h * NF:(h + 1) * NF]
                    )
                mv = stat.tile([P, 2], F32, tag=f"mv{tag}")
                nc.vector.bn_aggr(out=mv, in_=st)
                rstd = stat.tile([P, 1], F32, tag=f"rs{tag}")
                nc.scalar.activation(
                    out=rstd, in_=mv[:, 1:2], func=AF.Sqrt, bias=eps_t, scale=1.0
                )
                nc.vector.reciprocal(out=rstd, in_=rstd)
                return mv, rstd

            # ---- front half of tile j: x DMA, CxT, AO, r1, LN1, N1q, s ----
            def front(j):
                jsl = slice(j * P, (j + 1) * P)
                xb_t = xpool.tile([P, D], BF16, tag="xb", name="xb")
                nc.sync.dma_start(out=xb_t, in_=xb[jsl, :])
                xf_t = xpool.tile([P, D], F32, tag="xf", name="xf")
                nc.sync.dma_start(out=xf_t, in_=xf[jsl, :])

                # CxT blocks: [d-in-block, t] = prefix-mean^T, two 4-chunk halves
                cxq = qpool.tile([P, KC, P], F8E5, tag="cxq", name="cxq")
                for g in range(2):
                    ps_cx = pbig.tile([P, KC // 2, P], F32, tag="big")
                    for k4 in range(KC // 2):
                        kc = g * 4 + k4
                        nc.tensor.matmul(
                            ps_cx[:, k4, :],
                            lhsT=xb_t[:, kc * P:(kc + 1) * P],
                            rhs=masks_sb[:, j, :],
                            start=True, stop=True,
                        )
                    nc.scalar.activation(
                        out=cxq[:, g * 4:(g + 1) * 4, :], in_=ps_cx,
                        func=AF.Copy, scale=1.0,
                    )

                # AO = rank-2 (carry,bo) + CxT.T @ Wvo  (fp8 DoubleRow)
                r1 = fpool.tile([P, D], F32, tag="r1", name="r1")
                for n in range(NH):
                    nsl = slice(n * NF, (n + 1) * NF)
                    ps = pmm.tile([P, NF], F32, tag="mm")
                    nc.tensor.matmul(
                        ps, lhsT=cicn_sb[:, j, :],
                        rhs=crhs_sb[:, j, nsl],
                        start=True, stop=False,
                    )
                    for k2 in range(KC // 2):
                        nc.tensor.matmul(
                            ps,
                            lhsT=cxq[:, 2 * k2:2 * k2 + 2, :],
                            rhs=Wvo_sb[:, 2 * k2:2 * k2 + 2, nsl],
                            start=False, stop=(k2 == KC // 2 - 1),
                            perf_mode=DR,
                        )
                    nc.vector.scalar_tensor_tensor(
                        out=r1[:, nsl], in0=ps, scalar=1.0 / SW,
                        in1=xf_t[:, nsl], op0=OP.mult, op1=OP.add,
                    )

                mv1, rstd1 = stats(r1, "1")
                n1q = qpool.tile([P, D], BF16, tag="n1q", name="n1q")
                s = fpool.tile([P, D], F32, tag="s", name="s")
                if lean:
                    rstd16 = stat.tile([P, 1], F32, tag="r16")
                    nc.vector.tensor_scalar_mul(
                        out=rstd16, in0=rstd1, scalar1=SN
                    )
                    mb16 = stat.tile([P, 1], F32, tag="mb16")
                    nc.vector.tensor_scalar(
                        out=mb16, in0=mv1[:, 0:1], scalar1=rstd1,
                        scalar2=-SN, op0=OP.mult, op1=OP.mult,
                    )
                    nc.scalar.activation(
                        out=n1q, in_=r1, func=AF.Identity,
                        bias=mb16, scale=rstd16,
                    )
                    # s = r1*rstd + x == N1 + x + mean*rstd; the per-row
                    # constant cancels in LN2
                    nc.vector.scalar_tensor_tensor(
                        out=s, in0=r1, scalar=rstd1, in1=xf_t,
                        op0=OP.mult, op1=OP.add,
                    )
                else:
                    mb = stat.tile([P, 1], F32, tag="mb")
                    nc.vector.tensor_scalar(
                        out=mb, in0=mv1[:, 0:1], scalar1=rstd1,
                        scalar2=-1.0, op0=OP.mult, op1=OP.mult,
                    )
                    n1f = fpool.tile([P, D], F32, tag="n1f", name="n1f")
                    nc.scalar.activation(
                        out=n1f, in_=r1, func=AF.Identity, bias=mb, scale=rstd1
                    )
                    nc.vector.tensor_mul(out=n1f, in0=n1f, in1=bc["g1"])
                    nc.vector.tensor_add(out=n1f, in0=n1f, in1=bc["b1"])
                    nc.scalar.activation(
                        out=n1q, in_=n1f, func=AF.Copy, scale=SN
                    )
                    nc.vector.tensor_add(out=s, in0=n1f, in1=xf_t)
                return n1q, s

            # ---- back half of tile j: N1T, H^T, z, LN2, out ----
            def back(j, n1q, s):
                jsl = slice(j * P, (j + 1) * P)
                ps_nt = ptp.tile([P, KC, P], BF16, tag="tp")
                for kc in range(KC):
                    nc.tensor.transpose(
                        ps_nt[:, kc, :], n1q[:, kc * P:(kc + 1) * P], identb
                    )
                n1t = qpool.tile([P, KC, P], F8E4, tag="n1t", name="n1t")
                nc.scalar.activation(
                    out=n1t, in_=ps_nt, func=AF.Copy, scale=1.0
                )

                # H^T[f-in-block, t] per f-block, Wf1 stationary
                hqT = qpool.tile([P, KC, P], F8E4, tag="hqT", name="hqT")
                for g in range(2):
                    ps_ht = pbig.tile([P, KC // 2, P], F32, tag="big")
                    for f4 in range(KC // 2):
                        fb = g * 4 + f4
                        fsl = slice(fb * P, (fb + 1) * P)
                        for k2 in range(KC // 2):
                            nc.tensor.matmul(
                                ps_ht[:, f4, :],
                                lhsT=Wf1_sb[:, 2 * k2:2 * k2 + 2, fsl],
                                rhs=n1t[:, 2 * k2:2 * k2 + 2, :],
                                start=(k2 == 0), stop=(k2 == KC // 2 - 1),
                                perf_mode=DR,
                            )
                    if lean:
                        nc.scalar.activation(
                            out=hqT[:, g * 4:(g + 1) * 4, :], in_=ps_ht,
                            func=AF.Relu, scale=SH / (SN * SW),
                        )
                    else:
                        # SH*relu(H/(SN*SW) + bf1): bias = bf1*SH per f
                        for f4 in range(KC // 2):
                            fb = g * 4 + f4
                            nc.scalar.activation(
                                out=hqT[:, fb, :], in_=ps_ht[:, f4, :],
                                func=AF.Relu, scale=SH / (SN * SW),
                                bias=bf1T_sb[:, fb:fb + 1],
                            )

                zin = fpool.tile([P, D], F32, tag="z", name="z")
                for n in range(NH):
                    nsl = slice(n * NF, (n + 1) * NF)
                    ps = pmm.tile([P, NF], F32, tag="mm")
                    for k2 in range(KC // 2):
                        nc.tensor.matmul(
                            ps,
                            lhsT=hqT[:, 2 * k2:2 * k2 + 2, :],
                            rhs=Wf2_sb[:, 2 * k2:2 * k2 + 2, nsl],
                            start=(k2 == 0), stop=(k2 == KC // 2 - 1),
                            perf_mode=DR,
                        )
                    nc.vector.scalar_tensor_tensor(
                        out=zin[:, nsl], in0=ps, scalar=1.0 / (SH * SW),
                        in1=s[:, nsl], op0=OP.mult, op1=OP.add,
                    )
                if not lean:
                    nc.vector.tensor_add(out=zin, in0=zin, in1=bc["bf2"])

                mv2, rstd2 = stats(zin, "2")
                mb2 = stat.tile([P, 1], F32, tag="mb2")
                nc.vector.tensor_scalar(
                    out=mb2, in0=mv2[:, 0:1], scalar1=rstd2,
                    scalar2=-1.0, op0=OP.mult, op1=OP.mult,
                )
                o = fpool.tile([P, D], F32, tag="o", name="o")
                nc.scalar.activation(
                    out=o, in_=zin, func=AF.Identity, bias=mb2, scale=rstd2
                )
                if not lean:
                    nc.vector.tensor_mul(out=o, in0=o, in1=bc["g2"])
                    nc.vector.tensor_add(out=o, in0=o, in1=bc["b2"])
                nc.sync.dma_start(out=out[jsl, :], in_=o)

            # software pipeline: front(j+1) is emitted before back(j) so
            # the PE always has independent work while LN chains resolve
            pend = front(0)
            for j in range(NT):
                nxt = front(j + 1) if j + 1 < NT else None
                back(j, *pend)
                pend = nxt

    nc.compile()
    return nc


_CACHE = {}


def _get_nc(lean=True):
    key = "lean" if lean else "general"
    if key not in _CACHE:
        _CACHE[key] = _build(lean=lean)
    return _CACHE[key]


def _e4(a):
    return np.asarray(a, np.float32).astype(ml_dtypes.float8_e4m3)


def _bf(a):
    return np.asarray(a, np.float32).astype(ml_dtypes.bfloat16)


def _wlayout(w):
    """[D, D] -> [P, KC, D] with element (p, kc, n) = w[kc*P+p, n]."""
    return np.ascontiguousarray(
        np.asarray(w).reshape(KC, P, D).transpose(1, 0, 2)
    )


def _in_maps(x, Wv, Wo, bo, g1, b1, Wf1, bf1, Wf2, bf2, g2, b2):
    x = np.asarray(x, dtype=np.float32)
    Wv_all = np.asarray(Wv, np.float32).transpose(1, 0, 2).reshape(D, D)
    W_vo = Wv_all @ np.asarray(Wo, np.float32)
    base = {
        "Wvo": _wlayout(_e4(W_vo * SW)),
        "Wf1": _wlayout(_e4(np.asarray(Wf1, np.float32) * SW)),
        "Wf2": _wlayout(_e4(np.asarray(Wf2, np.float32) * SW)),
    }
    bo_v = np.asarray(bo, np.float32)

    # per-half masks and carry tables
    # block colsums: cs[b, k] = sum of x[b, :k*P] rows (k = 0..16)
    cs = np.zeros((B, 2 * NT + 1, D), np.float32)
    cs[:, 1:] = np.cumsum(x.reshape(B, 2 * NT, P, D).sum(2), axis=1)

    in_maps = []
    for c in range(8):
        b, half = divmod(c, 2)
        t0b = half * TH
        m = dict(base)
        m["xf"] = np.ascontiguousarray(x[b, t0b:t0b + TH])
        m["xb"] = _bf(m["xf"])
        tl = np.arange(P, dtype=np.float64)
        masks = np.empty((NT, P, P), np.float32)
        cicn = np.empty((2, NT, P), np.float32)
        crhs = np.empty((2, NT, D), np.float32)
        for j in range(NT):
            t0 = t0b + j * P
            icnt = (1.0 / (t0 + tl + 1.0)).astype(np.float32)
            masks[j] = (tl[:, None] <= tl[None, :]) * icnt[None, :]
            cicn[0, j] = icnt
            cicn[1, j] = 1.0
            carry = cs[b, half * NT + j]
            crhs[0, j] = (carry @ W_vo) * SW
            crhs[1, j] = bo_v * SW
        m["masks"] = _bf(masks)
        m["cicn"] = _bf(cicn)
        m["crhs"] = _bf(crhs)
        in_maps.append(m)
    return in_maps


def _in_maps_general(g1, b1, bf1, bf2, g2, b2):
    return {
        "g1": np.asarray(g1, np.float32).reshape(1, D),
        "b1": np.asarray(b1, np.float32).reshape(1, D),
        "bf1T": np.ascontiguousarray(
            np.asarray(bf1, np.float32).reshape(KC, P).T * SH
        ),
        "bf2": np.asarray(bf2, np.float32).reshape(1, D),
        "g2": np.asarray(g2, np.float32).reshape(1, D),
        "b2": np.asarray(b2, np.float32).reshape(1, D),
    }


def _assemble(results):
    out = np.empty((B, T, D), np.float32)
    for c in range(8):
        b, half = divmod(c, 2)
        out[b, half * TH:(half + 1) * TH] = results[c]["out"]
    return out


def kernel(x, Wk, Wv, Wo, bo, g1, b1, Wf1, bf1, Wf2, bf2, g2, b2):
    lean = bool(
        not np.any(np.asarray(b1)) and not np.any(np.asarray(bf1))
        and not np.any(np.asarray(bf2)) and not np.any(np.asarray(b2))
        and np.all(np.asarray(g1) == 1.0) and np.all(np.asarray(g2) == 1.0)
    )
    in_maps = _in_maps(x, Wv, Wo, bo, g1, b1, Wf1, bf1, Wf2, bf2, g2, b2)
    if not lean:
        extra = _in_maps_general(g1, b1, bf1, bf2, g2, b2)
        for m in in_maps:
            m.update(extra)
    res = run_bass_kernel_spmd(_get_nc(lean), in_maps, list(range(8))).results
    return _assemble(res)


# revision 22
# speedup vs baseline: 2.7389x; 1.0082x over previous
"""TRN2 8-core SPMD kernel for nn_DecoderBlock_13443247636967.

Math note (validated to rel err ~1.3e-7 against the fp32 reference):
the reference uses SCALE = head_size**-5 = 2**-30, so every pre-softmax
score satisfies |s| < 4e-8 and the softmax IS the uniform causal
average at fp32 precision.  Attention therefore reduces to a causal
prefix-mean of V, and since prefix-mean commutes with the projections,
AO = prefix_mean(x) @ (Wv_all @ Wo) -- ONE fused [D,D] matrix W_vo
(host-precomputed), applied to causal prefix means of x directly.

Device pipeline per 128-row tile (all GEMMs fp8 DoubleRow, PSUM fp32):
  CxT   = x_tile^T @ (tril*icnt mask)      bf16 PE matmuls (fused
          prefix-mean + transpose in one op)
  AO    = CxT^T @ W_vo_fp8 + rank-2 carry  e5m2 x e4m3 DoubleRow;
          the prefix carry (host colsums @ W_vo) and bo enter as a
          K=2 matmul (icnt/ones rows x cvo/bo rows) into the PSUM
  r1    = AO/SW + x ; LN1 -> N1q (fp8, x16)
  s     = r1*rstd + x  (the -mean*rstd per-row constant provably
          cancels inside LN2, so fp32 N1 is never materialized)
  N1T   = PE transpose of N1q (fp8)
  H^T   = Wf1^T-stationary @ N1T           e4m3 DoubleRow (computing H
          transposed kills the second transpose); relu+quant -> hqT
  z     = hqT^T @ Wf2 / (SH*SW) + s ; LN2 -> out

Sharding: core c = (batch c//2, half c%2) owns 1024 rows, data
parallel; no collectives (carries are host-side prefix colsums).

Emission is software-pipelined (tile j+1 front half before tile j back
half) so the tensor engine never drains and holds its max p-state.
Measured accuracy of this quantization scheme (host sim): 7.6e-3.
"""

import numpy as np
import ml_dtypes

import concourse.bass as bass
import concourse.mybir as mybir
import concourse.tile as tile
from concourse import bacc
from concourse.bass_utils import run_bass_kernel_spmd
from concourse.masks import make_identity

P = 128          # partitions / row-tile height
D = 1024         # model dim
TH = 1024        # sequence rows per core
NT = TH // P     # 8 row tiles
KC = D // P      # 8 contraction chunks
NF = 512         # PSUM half width
NH = D // NF     # 2 column halves
B, T = 4, 2048
EPS = 1e-5
SW = 1024.0      # weight fp8 scale (2**10)
SN = 16.0        # N1 fp8 scale (2**4)
SH = 32.0        # h fp8 scale (2**5)
F32 = mybir.dt.float32
BF16 = mybir.dt.bfloat16
F8E4 = mybir.dt.float8e4
F8E5 = mybir.dt.float8e5
DR = mybir.MatmulPerfMode.DoubleRow
AF = mybir.ActivationFunctionType
OP = mybir.AluOpType


def _build(lean=True):
    nc = bacc.Bacc(
        "TRN2", target_bir_lowering=False, debug=False, num_devices=8
    )
    xf = nc.dram_tensor("xf", [TH, D], F32, kind="ExternalInput").ap()
    xb = nc.dram_tensor("xb", [TH, D], BF16, kind="ExternalInput").ap()
    masks = nc.dram_tensor("masks", [NT, P, P], BF16, kind="ExternalInput").ap()
    cicn = nc.dram_tensor("cicn", [2, NT, P], BF16, kind="ExternalInput").ap()
    crhs = nc.dram_tensor("crhs", [2, NT, D], BF16, kind="ExternalInput").ap()
    Wvo = nc.dram_tensor("Wvo", [P, KC, D], F8E4, kind="ExternalInput").ap()
    Wf1 = nc.dram_tensor("Wf1", [P, KC, D], F8E4, kind="ExternalInput").ap()
    Wf2 = nc.dram_tensor("Wf2", [P, KC, D], F8E4, kind="ExternalInput").ap()
    out = nc.dram_tensor("out", [TH, D], F32, kind="ExternalOutput").ap()
    if not lean:
        vecs = {
            name: nc.dram_tensor(name, [1, D], F32, kind="ExternalInput").ap()
            for name in ["g1", "b1", "bf2", "g2", "b2"]
        }
        bf1T = nc.dram_tensor("bf1T", [P, KC], F32, kind="ExternalInput").ap()

    with tile.TileContext(nc) as tc:
        with tc.tile_pool(name="rows", bufs=1) as rows, \
             tc.tile_pool(name="w", bufs=1) as wpool, \
             tc.tile_pool(name="x", bufs=3) as xpool, \
             tc.tile_pool(name="q", bufs=2) as qpool, \
             tc.tile_pool(name="f", bufs=2) as fpool, \
             tc.tile_pool(name="st", bufs=3) as stat, \
             tc.tile_pool(name="pb", bufs=4, space="PSUM") as pbig, \
             tc.tile_pool(name="pm", bufs=3, space="PSUM") as pmm, \
             tc.tile_pool(name="pt", bufs=1, space="PSUM") as ptp:

            identb = rows.tile([P, P], BF16)
            make_identity(nc, identb)
            eps_t = rows.tile([P, 1], F32)
            nc.vector.memset(eps_t, EPS)
            # constants + weights go on the gpsimd DMA queue so the
            # streaming x tiles (sync queue) are not stuck behind 3MB
            masks_sb = rows.tile([P, NT, P], BF16)
            nc.gpsimd.dma_start(out=masks_sb, in_=masks.rearrange("j p t -> p j t"))
            cicn_sb = rows.tile([2, NT, P], BF16)
            nc.gpsimd.dma_start(out=cicn_sb, in_=cicn)
            crhs_sb = rows.tile([2, NT, D], BF16)
            nc.gpsimd.dma_start(out=crhs_sb, in_=crhs)
            Wvo_sb = wpool.tile([P, KC, D], F8E4, name="Wvo")
            nc.gpsimd.dma_start(out=Wvo_sb, in_=Wvo)
            Wf1_sb = wpool.tile([P, KC, D], F8E4, name="Wf1")
            nc.gpsimd.dma_start(out=Wf1_sb, in_=Wf1)
            Wf2_sb = wpool.tile([P, KC, D], F8E4, name="Wf2")
            nc.gpsimd.dma_start(out=Wf2_sb, in_=Wf2)
            if not lean:
                bc = {
                    name: rows.tile([P, D], F32, name=f"bc_{name}")
                    for name in vecs
                }
                for name in vecs:
                    nc.sync.dma_start(
                        out=bc[name], in_=vecs[name].to_broadcast([P, D])
                    )
                bf1T_sb = rows.tile([P, KC], F32)
                nc.sync.dma_start(out=bf1T_sb, in_=bf1T)

            def stats(src, tag):
                st = stat.tile([P, NH, 6], F32, tag=f"st{tag}")
                for h in range(NH):
                    nc.vector.bn_stats(
                        out=st[:, h, :], in_=src[:, h * NF:(h + 1) * NF]
                    )
                mv = stat.tile([P, 2], F32, tag=f"mv{tag}")
                nc.vector.bn_aggr(out=mv, in_=st)
                rstd = stat.tile([P, 1], F32, tag=f"rs{tag}")
                nc.scalar.activation(
                    out=rstd, in_=mv[:, 1:2], func=AF.Sqrt, bias=eps_t, scale=1.0
                )
                nc.vector.reciprocal(out=rstd, in_=rstd)
                return mv, rstd

            # ---- stage A of tile j: x DMA, CxT, AO, r1, LN1 stats ----
            def stageA(j):
                jsl = slice(j * P, (j + 1) * P)
                xb_t = xpool.tile([P, D], BF16, tag="xb", name="xb")
                nc.sync.dma_start(out=xb_t, in_=xb[jsl, :])
                xf_t = xpool.tile([P, D], F32, tag="xf", name="xf")
                nc.sync.dma_start(out=xf_t, in_=xf[jsl, :])

                # CxT blocks: [d-in-block, t] = prefix-mean^T, two 4-chunk halves
                cxq = qpool.tile([P, KC, P], F8E5, tag="cxq", name="cxq")
                for g in range(2):
                    ps_cx = pbig.tile([P, KC // 2, P], F32, tag="big")
                    for k4 in range(KC // 2):
                        kc = g * 4 + k4
                        nc.tensor.matmul(
                            ps_cx[:, k4, :],
                            lhsT=xb_t[:, kc * P:(kc + 1) * P],
                            rhs=masks_sb[:, j, :],
                            start=True, stop=True,
                        )
                    nc.scalar.activation(
                        out=cxq[:, g * 4:(g + 1) * 4, :], in_=ps_cx,
                        func=AF.Copy, scale=1.0,
                    )

                # AO = rank-2 (carry,bo) + CxT.T @ Wvo  (fp8 DoubleRow)
                r1 = fpool.tile([P, D], F32, tag="r1", name="r1")
                for n in range(NH):
                    nsl = slice(n * NF, (n + 1) * NF)
                    ps = pmm.tile([P, NF], F32, tag="mm")
                    nc.tensor.matmul(
                        ps, lhsT=cicn_sb[:, j, :],
                        rhs=crhs_sb[:, j, nsl],
                        start=True, stop=False,
                    )
                    for k2 in range(KC // 2):
                        nc.tensor.matmul(
                            ps,
                            lhsT=cxq[:, 2 * k2:2 * k2 + 2, :],
                            rhs=Wvo_sb[:, 2 * k2:2 * k2 + 2, nsl],
                            start=False, stop=(k2 == KC // 2 - 1),
                            perf_mode=DR,
                        )
                    nc.vector.scalar_tensor_tensor(
                        out=r1[:, nsl], in0=ps, scalar=1.0 / SW,
                        in1=xf_t[:, nsl], op0=OP.mult, op1=OP.add,
                    )

                mv1, rstd1 = stats(r1, "1")
                return r1, xf_t, mv1, rstd1

            # ---- stage B of tile j: N1q write + residual s ----
            def stageB(j, actx):
                r1, xf_t, mv1, rstd1 = actx
                n1q = qpool.tile([P, D], BF16, tag="n1q", name="n1q")
                s = fpool.tile([P, D], F32, tag="s", name="s")
                if lean:
                    rstd16 = stat.tile([P, 1], F32, tag="r16")
                    nc.vector.tensor_scalar_mul(
                        out=rstd16, in0=rstd1, scalar1=SN
                    )
                    mb16 = stat.tile([P, 1], F32, tag="mb16")
                    nc.vector.tensor_scalar(
                        out=mb16, in0=mv1[:, 0:1], scalar1=rstd1,
                        scalar2=-SN, op0=OP.mult, op1=OP.mult,
                    )
                    nc.scalar.activation(
                        out=n1q, in_=r1, func=AF.Identity,
                        bias=mb16, scale=rstd16,
                    )
                    # s = r1*rstd + x == N1 + x + mean*rstd; the per-row
                    # constant cancels in LN2
                    nc.vector.scalar_tensor_tensor(
                        out=s, in0=r1, scalar=rstd1, in1=xf_t,
                        op0=OP.mult, op1=OP.add,
                    )
                else:
                    mb = stat.tile([P, 1], F32, tag="mb")
                    nc.vector.tensor_scalar(
                        out=mb, in0=mv1[:, 0:1], scalar1=rstd1,
                        scalar2=-1.0, op0=OP.mult, op1=OP.mult,
                    )
                    n1f = fpool.tile([P, D], F32, tag="n1f", name="n1f")
                    nc.scalar.activation(
                        out=n1f, in_=r1, func=AF.Identity, bias=mb, scale=rstd1
                    )
                    nc.vector.tensor_mul(out=n1f, in0=n1f, in1=bc["g1"])
                    nc.vector.tensor_add(out=n1f, in0=n1f, in1=bc["b1"])
                    nc.scalar.activation(
                        out=n1q, in_=n1f, func=AF.Copy, scale=SN
                    )
                    nc.vector.tensor_add(out=s, in0=n1f, in1=xf_t)
                return n1q, s

            # ---- stage C of tile j: N1T, H^T, z, LN2, out ----
            def stageC(j, n1q, s):
                jsl = slice(j * P, (j + 1) * P)
                ps_nt = ptp.tile([P, KC, P], BF16, tag="tp")
                for kc in range(KC):
                    nc.tensor.transpose(
                        ps_nt[:, kc, :], n1q[:, kc * P:(kc + 1) * P], identb
                    )
                n1t = qpool.tile([P, KC, P], F8E4, tag="n1t", name="n1t")
                nc.scalar.activation(
                    out=n1t, in_=ps_nt, func=AF.Copy, scale=1.0
                )

                # H^T[f-in-block, t] per f-block, Wf1 stationary
                hqT = qpool.tile([P, KC, P], F8E4, tag="hqT", name="hqT")
                for g in range(2):
                    ps_ht = pbig.tile([P, KC // 2, P], F32, tag="big")
                    for f4 in range(KC // 2):
                        fb = g * 4 + f4
                        fsl = slice(fb * P, (fb + 1) * P)
                        for k2 in range(KC // 2):
                            nc.tensor.matmul(
                                ps_ht[:, f4, :],
                                lhsT=Wf1_sb[:, 2 * k2:2 * k2 + 2, fsl],
                                rhs=n1t[:, 2 * k2:2 * k2 + 2, :],
                                start=(k2 == 0), stop=(k2 == KC // 2 - 1),
                                perf_mode=DR,
                            )
                    if lean:
                        nc.scalar.activation(
                            out=hqT[:, g * 4:(g + 1) * 4, :], in_=ps_ht,
                            func=AF.Relu, scale=SH / (SN * SW),
                        )
                    else:
                        # SH*relu(H/(SN*SW) + bf1): bias = bf1*SH per f
                        for f4 in range(KC // 2):
                            fb = g * 4 + f4
                            nc.scalar.activation(
                                out=hqT[:, fb, :], in_=ps_ht[:, f4, :],
                                func=AF.Relu, scale=SH / (SN * SW),
                                bias=bf1T_sb[:, fb:fb + 1],
                            )

                zin = fpool.tile([P, D], F32, tag="z", name="z")
                for n in range(NH):
                    nsl = slice(n * NF, (n + 1) * NF)
                    ps = pmm.tile([P, NF], F32, tag="mm")
                    for k2 in range(KC // 2):
                        nc.tensor.matmul(
                            ps,
                            lhsT=hqT[:, 2 * k2:2 * k2 + 2, :],
                            rhs=Wf2_sb[:, 2 * k2:2 * k2 + 2, nsl],
                            start=(k2 == 0), stop=(k2 == KC // 2 - 1),
                            perf_mode=DR,
                        )
                    nc.vector.scalar_tensor_tensor(
                        out=zin[:, nsl], in0=ps, scalar=1.0 / (SH * SW),
                        in1=s[:, nsl], op0=OP.mult, op1=OP.add,
                    )
                if not lean:
                    nc.vector.tensor_add(out=zin, in0=zin, in1=bc["bf2"])

                mv2, rstd2 = stats(zin, "2")
                mb2 = stat.tile([P, 1], F32, tag="mb2")
                nc.vector.tensor_scalar(
                    out=mb2, in0=mv2[:, 0:1], scalar1=rstd2,
                    scalar2=-1.0, op0=OP.mult, op1=OP.mult,
                )
                o = fpool.tile([P, D], F32, tag="o", name="o")
                nc.scalar.activation(
                    out=o, in_=zin, func=AF.Identity, bias=mb2, scale=rstd2
                )
                if not lean:
                    nc.vector.tensor_mul(out=o, in0=o, in1=bc["g2"])
                    nc.vector.tensor_add(out=o, in0=o, in1=bc["b2"])
                nc.sync.dma_start(out=out[jsl, :], in_=o)

            # software pipeline, emission order A(j+1), C(j), B(j+1):
            # in-order engine queues then never head-of-line block (the
            # ACT queue runs cx(j+1), n1t(j), hqT(j), o(j), n1q(j+1))
            # and the PE always has tile-j+1 matmuls while tile-j LN
            # chains resolve.
            bcur = stageB(0, stageA(0))
            for j in range(NT):
                anext = stageA(j + 1) if j + 1 < NT else None
                stageC(j, *bcur)
                bcur = stageB(j + 1, anext) if j + 1 < NT else None

    nc.compile()
    return nc


_CACHE = {}


def _get_nc(lean=True):
    key = "lean" if lean else "general"
    if key not in _CACHE:
        _CACHE[key] = _build(lean=lean)
    return _CACHE[key]


def _e4(a):
    return np.asarray(a, np.float32).astype(ml_dtypes.float8_e4m3)


def _bf(a):
    return np.asarray(a, np.float32).astype(ml_dtypes.bfloat16)


def _wlayout(w):
    """[D, D] -> [P, KC, D] with element (p, kc, n) = w[kc*P+p, n]."""
    return np.ascontiguousarray(
        np.asarray(w).reshape(KC, P, D).transpose(1, 0, 2)
    )


def _in_maps(x, Wv, Wo, bo, g1, b1, Wf1, bf1, Wf2, bf2, g2, b2):
    x = np.asarray(x, dtype=np.float32)
    Wv_all = np.asarray(Wv, np.float32).transpose(1, 0, 2).reshape(D, D)
    W_vo = Wv_all @ np.asarray(Wo, np.float32)
    base = {
        "Wvo": _wlayout(_e4(W_vo * SW)),
        "Wf1": _wlayout(_e4(np.asarray(Wf1, np.float32) * SW)),
        "Wf2": _wlayout(_e4(np.asarray(Wf2, np.float32) * SW)),
    }
    bo_v = np.asarray(bo, np.float32)

    # per-half masks and carry tables
    # block colsums: cs[b, k] = sum of x[b, :k*P] rows (k = 0..16)
    cs = np.zeros((B, 2 * NT + 1, D), np.float32)
    cs[:, 1:] = np.cumsum(x.reshape(B, 2 * NT, P, D).sum(2), axis=1)

    in_maps = []
    for c in range(8):
        b, half = divmod(c, 2)
        t0b = half * TH
        m = dict(base)
        m["xf"] = np.ascontiguousarray(x[b, t0b:t0b + TH])
        m["xb"] = _bf(m["xf"])
        tl = np.arange(P, dtype=np.float64)
        masks = np.empty((NT, P, P), np.float32)
        cicn = np.empty((2, NT, P), np.float32)
        crhs = np.empty((2, NT, D), np.float32)
        for j in range(NT):
            t0 = t0b + j * P
            icnt = (1.0 / (t0 + tl + 1.0)).astype(np.float32)
            masks[j] = (tl[:, None] <= tl[None, :]) * icnt[None, :]
            cicn[0, j] = icnt
            cicn[1, j] = 1.0
            carry = cs[b, half * NT + j]
            crhs[0, j] = (carry @ W_vo) * SW
            crhs[1, j] = bo_v * SW
        m["masks"] = _bf(masks)
        m["cicn"] = _bf(cicn)
        m["crhs"] = _bf(crhs)
        in_maps.append(m)
    return in_maps


def _in_maps_general(g1, b1, bf1, bf2, g2, b2):
    return {
        "g1": np.asarray(g1, np.float32).reshape(1, D),
        "b1": np.asarray(b1, np.float32).reshape(1, D),
        "bf1T": np.ascontiguousarray(
            np.asarray(bf1, np.float32).reshape(KC, P).T * SH
        ),
        "bf2": np.asarray(bf2, np.float32).reshape(1, D),
        "g2": np.asarray(g2, np.float32).reshape(1, D),
        "b2": np.asarray(b2, np.float32).reshape(1, D),
    }


def _assemble(results):
    out = np.empty((B, T, D), np.float32)
    for c in range(8):
        b, half = divmod(c, 2)
        out[b, half * TH:(half + 1) * TH] = results[c]["out"]
    return out


def kernel(x, Wk, Wv, Wo, bo, g1, b1, Wf1, bf1, Wf2, bf2, g2, b2):
    lean = bool(
        not np.any(np.asarray(b1)) and not np.any(np.asarray(bf1))
        and not np.any(np.asarray(bf2)) and not np.any(np.asarray(b2))
        and np.all(np.asarray(g1) == 1.0) and np.all(np.asarray(g2) == 1.0)
    )
    in_maps = _in_maps(x, Wv, Wo, bo, g1, b1, Wf1, bf1, Wf2, bf2, g2, b2)
    if not lean:
        extra = _in_maps_general(g1, b1, bf1, bf2, g2, b2)
        for m in in_maps:
            m.update(extra)
    res = run_bass_kernel_spmd(_get_nc(lean), in_maps, list(range(8))).results
    return _assemble(res)
